# revision 34
# baseline (speedup 1.0000x reference)
"""Trainium2 Bass kernel for nn_Criterion_32830730011569.

8 cores = (image b in 0..3) x (H-half h in 0..1). Host-side prep is pure
indexing/layout (slice, transpose, channel-gather by matched_q/matched_e,
pack) — all arithmetic runs on device.

Each core streams its [96,192] pixel slice once as a packed
[NCHUNK*P, JC, 192] tensor (matched-por || matched-true, chunk-contiguous
1.42MB linear DMAs, staggered prefetch):
  - dice: exp on ACT; softmax denominator is one DVE reduce (channels are
    pre-gathered so no mask is needed); bf16 matmuls accumulate
    C[m1,m2] = (true_r/Z)^T exp_r into PSUM. num = 2*trace(C); den =
    sum of all C entries (each row of exp_r/Z sums to 1).
  - 7x7-window BCE: true/bin shipped channel-major + padded so each m's
    whole window lives in one contiguous 1159-float run; ONE indirect DMA
    per tensor gathers all 96 runs; BCE uses ACT Softplus.
  - occupancy CE: streamed exp + ACT Ln logsumexp + label select.
  - class / NLL: tiny per-query math from host-gathered rows, ACT Ln.
Small setup inputs ride in one packed [128,SC] tensor on the Scalar
engine's HWDGE queue. Each core returns 7 partial sums; the host combines.
"""
import sys

sys.path.insert(0, "/opt/trn_rl_repo")
import numpy as np

B, H, W, Q, E, M, K, WIN = 4, 192, 192, 160, 96, 96, 4, 7
NO_E = 0.1
HALF = H // 2          # rows per core slice
NPIX = HALF * W        # 18432 pixels per slice
P = 128                # partitions
J = NPIX // P          # 144 pixels per partition
NCHUNK = 8
JC = J // NCHUNK       # 18
CH = 2 * M             # 192 packed channels (por_r | true_r)
MAGIC = 8388608.0      # 2^23
MAGIC_I = 0x4B000000
RUNW = 6 * W + 7       # one contiguous span covering a whole 7x7 window
PADF = RUNW            # front pad so straddle-up windows stay row-aligned
PADB = 2400

# smalls column map
C_RB, C_INC, C_DROF, C_POS, C_CHOL, C_MENP, C_MQNP = 0, 1, 3, 10, 12, 16, 17
C_IEL, C_IND, C_I96 = 18, 178, 338
SC = 338 + M

_CACHE = {}


def _build_nc():
    import concourse.bass as bass
    import concourse.bacc as bacc
    import concourse.tile as tile
    from concourse import mybir
    from concourse.tile import add_dep_helper

    f32 = mybir.dt.float32
    i32 = mybir.dt.int32
    bf16 = mybir.dt.bfloat16
    AF = mybir.ActivationFunctionType
    OP = mybir.AluOpType
    AX = mybir.AxisListType

    nc = bacc.Bacc("TRN2", target_bir_lowering=False, debug=False, num_devices=8)

    # ---- external I/O ----
    por_pack = nc.dram_tensor("por_pack", [NCHUNK * P, JC, M], f32, kind="ExternalInput")
    tru_pack = nc.dram_tensor("tru_pack", [NCHUNK * P, JC, M], f32, kind="ExternalInput")
    occ_pack = nc.dram_tensor("occ_pack", [P, J, K + 1], f32, kind="ExternalInput")
    true_cm = nc.dram_tensor("true_cm", [1, PADF + E * NPIX + PADB], f32, kind="ExternalInput")
    bin_cm = nc.dram_tensor("bin_cm", [1, PADF + Q * NPIX + PADB], f32, kind="ExternalInput")
    smalls = nc.dram_tensor("smalls", [P, SC], f32, kind="ExternalInput")
    partials = nc.dram_tensor("partials", [1, 8], f32, kind="ExternalOutput")

    def bc(ap, pos, count):
        """Insert a stride-0 broadcast dim into an AP at free-dim position pos."""
        new = list(ap.ap)
        new.insert(pos, [0, count])
        return bass.AP(tensor=ap.tensor, offset=ap.offset, ap=new)

    from contextlib import ExitStack

    with tile.TileContext(nc) as tc, ExitStack() as ctx:
        sing = ctx.enter_context(tc.tile_pool(name="sing", bufs=1))
        pkp = ctx.enter_context(tc.tile_pool(name="pkp", bufs=3))
        big = ctx.enter_context(tc.tile_pool(name="big", bufs=2))
        ps = ctx.enter_context(tc.tile_pool(name="ps", bufs=1, space="PSUM"))

        # ---------- DMA issues: chunk stream on Sync, smalls/occ on Scalar ----------
        def issue_chunk(c):
            # por on the Sync HWDGE queue set, true on the Scalar set: the two
            # 0.71MB transfers ride different queue sets in parallel
            tp = pkp.tile([P, JC, M], f32, tag="pkp")
            nc.sync.dma_start(out=tp[:], in_=por_pack.ap()[c * P:(c + 1) * P, :, :])
            tt = pkp.tile([P, JC, M], f32, tag="pkt")
            nc.scalar.dma_start(out=tt[:], in_=tru_pack.ap()[c * P:(c + 1) * P, :, :])
            return (tp, tt)

        pk_fifo = [issue_chunk(0)]
        sm = sing.tile([P, SC], f32)
        nc.scalar.dma_start(out=sm[:], in_=smalls.ap())
        occ_t = sing.tile([P, J, K + 1], f32)
        nc.scalar.dma_start(out=occ_t[:], in_=occ_pack.ap())
        pk_fifo.append(issue_chunk(1))

        def S(p0, p1, c0, c1):
            return sm[p0:p1, c0:c1]

        ones = sing.tile([P, 1], f32)
        nc.vector.memset(ones[:], 1.0)
        stats = sing.tile([P, 6], f32)
        nc.vector.memset(stats[:], 0.0)
        res = sing.tile([1, 8], f32)
        nc.vector.memset(res[:], 0.0)

        # ---------- window offsets (first DVE work; needs only smalls) ----------
        ptsr = S(0, M, C_INC, C_INC + 2)
        rmag = sing.tile([M, 2], f32)
        nc.vector.tensor_scalar(out=rmag[:], in0=ptsr, scalar1=MAGIC, scalar2=-MAGIC,
                                op0=OP.add, op1=OP.add)
        gtm = sing.tile([M, 2], f32)
        nc.vector.tensor_tensor(out=gtm[:], in0=rmag[:], in1=ptsr, op=OP.is_gt)
        pixf = sing.tile([M, 2], f32)
        nc.vector.tensor_tensor(out=pixf[:], in0=rmag[:], in1=gtm[:], op=OP.subtract)
        base = sing.tile([M, 1], f32)
        nc.vector.tensor_scalar(out=base[:], in0=pixf[:, 0:1], scalar1=float(W),
                                scalar2=float(-3 * W - 3), op0=OP.mult, op1=OP.add)
        nc.vector.tensor_tensor(out=base[:], in0=base[:], in1=pixf[:, 1:2], op=OP.add)
        sofs = sing.tile([M, WIN], f32)
        nc.vector.tensor_scalar(out=sofs[:], in0=S(0, M, C_DROF, C_DROF + WIN),
                                scalar1=base[:], scalar2=S(0, M, C_RB, C_RB + 1),
                                op0=OP.add, op1=OP.add)
        v1 = sing.tile([M, WIN], f32)
        nc.vector.tensor_scalar(out=v1[:], in0=sofs[:], scalar1=0.0, scalar2=None, op0=OP.is_ge)
        v2 = sing.tile([M, WIN], f32)
        nc.vector.tensor_scalar(out=v2[:], in0=sofs[:], scalar1=float(NPIX - 1), scalar2=None, op0=OP.is_le)
        valid = sing.tile([M, WIN], f32)
        nc.vector.tensor_tensor(out=valid[:], in0=v1[:], in1=v2[:], op=OP.mult)
        # one offset per m: start of the contiguous RUNW-float span, clamped so
        # the span stays inside the padded flat tensor. Border margins mean
        # straddling windows are never clamped, so valid rows stay row-aligned.
        clam0 = sing.tile([M, 1], f32)
        nc.vector.tensor_scalar(out=clam0[:], in0=sofs[:, 0:1], scalar1=float(-(6 * W + 4)),
                                scalar2=float(NPIX - WIN), op0=OP.max, op1=OP.min)
        soft = sing.tile([M, 1], f32)
        nc.vector.tensor_scalar(out=soft[:], in0=clam0[:], scalar1=S(0, M, C_MENP, C_MENP + 1),
                                scalar2=MAGIC + PADF, op0=OP.add, op1=OP.add)
        soft_i = sing.tile([M, 1], i32)
        nc.vector.tensor_scalar(out=soft_i[:], in0=soft[:].bitcast(i32), scalar1=0x007FFFFF,
                                scalar2=None, op0=OP.bitwise_and)
        sofb = sing.tile([M, 1], f32)
        nc.vector.tensor_scalar(out=sofb[:], in0=clam0[:], scalar1=S(0, M, C_MQNP, C_MQNP + 1),
                                scalar2=MAGIC + PADF, op0=OP.add, op1=OP.add)
        sofb_i = sing.tile([M, 1], i32)
        nc.vector.tensor_scalar(out=sofb_i[:], in0=sofb[:].bitcast(i32), scalar1=0x007FFFFF,
                                scalar2=None, op0=OP.bitwise_and)

        # ---------- window gathers: one RUNW-float run per matched electron ----------
        tw = sing.tile([M, RUNW], f32)
        bw = sing.tile([M, RUNW], f32)
        true_flat = bass.AP(tensor=true_cm.ap().tensor, offset=0,
                            ap=[[1, PADF + E * NPIX + PADB], [1, 1]])
        bin_flat = bass.AP(tensor=bin_cm.ap().tensor, offset=0,
                           ap=[[1, PADF + Q * NPIX + PADB], [1, 1]])
        nc.gpsimd.indirect_dma_start(
            out=tw[:], out_offset=None, in_=true_flat,
            in_offset=bass.IndirectOffsetOnAxis(ap=soft_i[:], axis=0))
        nc.gpsimd.indirect_dma_start(
            out=bw[:], out_offset=None, in_=bin_flat,
            in_offset=bass.IndirectOffsetOnAxis(ap=sofb_i[:], axis=0))

        # ---------- dice streaming ----------
        C_ps = ps.tile([M, M], f32)
        for c in range(NCHUNK):
            pk_p, pk_tt = pk_fifo.pop(0)
            por_v = pk_p[:]
            tru_v = pk_tt[:]
            exp_t = big.tile([P, JC, M], bf16, tag="exp")
            exp_i = nc.scalar.activation(out=exp_t[:], in_=por_v, func=AF.Exp)
            if c == NCHUNK - 1:
                last_exp = exp_i
            zq_t = big.tile([P, JC], f32, tag="zq")
            nc.vector.reduce_sum(out=zq_t[:], in_=exp_t[:], axis=AX.X)
            rz_t = big.tile([P, JC], bf16, tag="rz")
            with nc.allow_low_precision(reason="rz scales both num and den; error cancels in dice ratio"):
                nc.vector.reciprocal(out=rz_t[:], in_=zq_t[:])
            tc_t = big.tile([P, JC, M], bf16, tag="tc")
            with nc.allow_low_precision(reason="bf16 matmul operand cast"):
                nc.gpsimd.tensor_copy(out=tc_t[:], in_=tru_v)
            a_t = big.tile([P, JC, M], bf16, tag="a")
            nc.vector.tensor_tensor(out=a_t[:], in0=tc_t[:], in1=bc(rz_t[:], 2, M), op=OP.mult)
            for kb in range(JC):
                nc.tensor.matmul(out=C_ps[:], lhsT=a_t[:, kb, :], rhs=exp_t[:, kb, :],
                                 start=(c == 0 and kb == 0),
                                 stop=(c == NCHUNK - 1 and kb == JC - 1))
            if c + 2 < NCHUNK:
                pk_fifo.append(issue_chunk(c + 2))

            # ---- Exp-table / DVE work slotted between chunks (Ln deferred) ----
            if c == 2:
                e4 = sing.tile([P, J, K], f32)
                nc.scalar.activation(out=e4[:], in_=occ_t[:, :, 0:K], func=AF.Exp)
                s4 = sing.tile([P, J], f32)
                nc.vector.reduce_sum(out=s4[:], in_=e4[:], axis=AX.X)
            if c == 3:
                # occupancy CE: label select
                xt = sing.tile([P, J], f32)
                mk = sing.tile([P, J], f32)
                pk2 = sing.tile([P, J], f32)
                for k in range(K):
                    nc.vector.tensor_scalar(out=mk[:], in0=occ_t[:, :, K], scalar1=float(k),
                                            scalar2=None, op0=OP.is_equal)
                    if k == 0:
                        nc.vector.tensor_tensor(out=xt[:], in0=mk[:], in1=occ_t[:, :, 0], op=OP.mult)
                    else:
                        nc.vector.tensor_tensor(out=pk2[:], in0=mk[:], in1=occ_t[:, :, k], op=OP.mult)
                        nc.vector.tensor_tensor(out=xt[:], in0=xt[:], in1=pk2[:], op=OP.add)
            if c == 4:
                # 7x7 window BCE: the 49 window values sit at run[a*W + b]
                def win_ap(t):
                    pdim = t[:].ap[0]
                    return bass.AP(tensor=t[:].tensor, offset=t[:].offset,
                                   ap=[pdim, [W, WIN], [1, WIN]])

                tv = sing.tile([M, WIN * WIN], f32)
                nc.vector.tensor_copy(out=tv[:].rearrange("m (a b) -> m a b", a=WIN),
                                      in_=win_ap(tw))
                lg = sing.tile([M, WIN * WIN], f32)
                nc.vector.tensor_copy(out=lg[:].rearrange("m (a b) -> m a b", a=WIN),
                                      in_=win_ap(bw))
                # softplus = Ln(1 + exp); the Ln half runs in the tail
                exw = sing.tile([M, WIN * WIN], f32)
                nc.scalar.activation(out=exw[:], in_=lg[:], func=AF.Exp)
                nc.vector.tensor_scalar(out=exw[:], in0=exw[:], scalar1=1.0, scalar2=None, op0=OP.add)
                prw = sing.tile([M, WIN * WIN], f32)
                nc.vector.tensor_tensor(out=prw[:], in0=lg[:], in1=tv[:], op=OP.mult)
            if c == 5:
                valid49 = sing.tile([M, WIN * WIN], f32)
                nc.vector.tensor_copy(out=valid49[:].rearrange("m (a b) -> m a b", a=WIN),
                                      in_=bc(valid[:], 2, WIN))
                iel = S(0, 1, C_IEL, C_IEL + Q)
                exc = sing.tile([1, Q], f32)
                nc.scalar.activation(out=exc[:], in_=iel, func=AF.Exp)
                nc.vector.tensor_scalar(out=exc[:], in0=exc[:], scalar1=1.0, scalar2=None, op0=OP.add)
            if c == 6:
                # NLL (96 partitions): everything except the Ln
                cenr = S(0, M, C_POS, C_POS + 2)
                chol0 = S(0, M, C_CHOL, C_CHOL + 1)
                chol1 = S(0, M, C_CHOL + 2, C_CHOL + 3)
                chol3 = S(0, M, C_CHOL + 3, C_CHOL + 4)
                d_ = sing.tile([M, 2], f32)
                nc.vector.tensor_tensor(out=d_[:], in0=ptsr, in1=cenr, op=OP.subtract)
                r00 = sing.tile([M, 1], f32)
                nc.vector.reciprocal(out=r00[:], in_=chol0)
                r11 = sing.tile([M, 1], f32)
                nc.vector.reciprocal(out=r11[:], in_=chol3)
                z0 = sing.tile([M, 1], f32)
                nc.vector.tensor_tensor(out=z0[:], in0=d_[:, 0:1], in1=r00[:], op=OP.mult)
                t1 = sing.tile([M, 1], f32)
                nc.vector.tensor_tensor(out=t1[:], in0=chol1, in1=z0[:], op=OP.mult)
                nc.vector.tensor_tensor(out=t1[:], in0=d_[:, 1:2], in1=t1[:], op=OP.subtract)
                z1 = sing.tile([M, 1], f32)
                nc.vector.tensor_tensor(out=z1[:], in0=t1[:], in1=r11[:], op=OP.mult)
                sq = sing.tile([M, 1], f32)
                nc.vector.tensor_tensor(out=sq[:], in0=z0[:], in1=z0[:], op=OP.mult)
                sq1 = sing.tile([M, 1], f32)
                nc.vector.tensor_tensor(out=sq1[:], in0=z1[:], in1=z1[:], op=OP.mult)
                nc.vector.tensor_tensor(out=sq[:], in0=sq[:], in1=sq1[:], op=OP.add)
                ldet = sing.tile([M, 1], f32)
                nc.vector.tensor_tensor(out=ldet[:], in0=chol0, in1=chol3, op=OP.mult)
                nc.vector.tensor_scalar(out=sq[:], in0=sq[:], scalar1=0.5,
                                        scalar2=float(np.log(2.0 * np.pi)), op0=OP.mult, op1=OP.add)

        # ---------- tail: all Ln work (one ACT table switch), pinned last ----------
        lse = sing.tile([P, J], f32)
        ln_i = nc.scalar.activation(out=lse[:], in_=s4[:], func=AF.Ln)
        add_dep_helper(ln_i.ins, last_exp.ins, reason="one table switch at tail")
        spw = sing.tile([M, WIN * WIN], f32)
        ln_w = nc.scalar.activation(out=spw[:], in_=exw[:], func=AF.Ln)
        add_dep_helper(ln_w.ins, last_exp.ins, reason="one table switch at tail")
        sp = sing.tile([1, Q], f32)
        ln_c = nc.scalar.activation(out=sp[:], in_=exc[:], func=AF.Ln)
        add_dep_helper(ln_c.ins, last_exp.ins, reason="one table switch at tail")
        lnd = sing.tile([M, 1], f32)
        ln_n = nc.scalar.activation(out=lnd[:], in_=ldet[:], func=AF.Ln)
        add_dep_helper(ln_n.ins, last_exp.ins, reason="one table switch at tail")

        # occupancy CE finish
        nc.vector.tensor_tensor(out=lse[:], in0=lse[:], in1=xt[:], op=OP.subtract)
        nc.vector.reduce_sum(out=stats[:, 4:5], in_=lse[:], axis=AX.X)
        # window BCE finish
        nc.vector.tensor_tensor(out=spw[:], in0=spw[:], in1=prw[:], op=OP.subtract)
        scr_w = sing.tile([M, WIN * WIN], f32)
        nc.vector.tensor_tensor(out=scr_w[:], in0=spw[:], in1=valid49[:], op=OP.mult)
        nc.vector.reduce_sum(out=stats[0:M, 1:2], in_=scr_w[:], axis=AX.X)
        # class finish
        iel = S(0, 1, C_IEL, C_IEL + Q)
        ind1 = S(0, 1, C_IND, C_IND + Q)
        t9 = sing.tile([1, Q], f32)
        nc.vector.tensor_scalar(out=t9[:], in0=sp[:], scalar1=0.9, scalar2=None, op0=OP.mult)
        nc.vector.tensor_tensor(out=t9[:], in0=t9[:], in1=iel, op=OP.subtract)
        scr_q = sing.tile([1, Q], f32)
        clsm = sing.tile([1, 1], f32)
        nc.vector.tensor_tensor(out=scr_q[:], in0=t9[:], in1=ind1, op=OP.mult)
        nc.vector.reduce_sum(out=clsm[:], in_=scr_q[:], axis=AX.X)
        spsum = sing.tile([1, 1], f32)
        nc.vector.reduce_sum(out=spsum[:], in_=sp[:], axis=AX.X)
        nc.vector.tensor_scalar(out=spsum[:], in0=spsum[:], scalar1=NO_E, scalar2=None, op0=OP.mult)
        nc.vector.tensor_tensor(out=res[:, 6:7], in0=spsum[:], in1=clsm[:], op=OP.add)
        # NLL finish
        nc.vector.tensor_tensor(out=stats[0:M, 0:1], in0=sq[:], in1=lnd[:], op=OP.add)

        # ---------- dice epilogue ----------
        Cs = sing.tile([M, M], f32)
        nc.vector.tensor_copy(out=Cs[:], in_=C_ps[:])
        # rhs rows (exp_r/Z) sum to 1, so summing all of C gives sum(true): den.
        nc.vector.reduce_sum(out=stats[0:M, 3:4], in_=Cs[:], axis=AX.X)
        scr_c = sing.tile([M, M], f32)
        nc.vector.tensor_tensor(out=scr_c[:], in0=Cs[:], in1=S(0, M, C_I96, C_I96 + M), op=OP.mult)
        nc.vector.reduce_sum(out=stats[0:M, 2:3], in_=scr_c[:], axis=AX.X)

        # ---------- final cross-partition reduction ----------
        fin_ps = ps.tile([1, 6], f32)
        nc.tensor.matmul(out=fin_ps[:], lhsT=ones[:], rhs=stats[:], start=True, stop=True)
        nc.vector.tensor_copy(out=res[:, 0:6], in_=fin_ps[:])
        nc.sync.dma_start(out=partials.ap(), in_=res[:])

    nc.compile()
    return nc


def _get_nc():
    if "nc" not in _CACHE:
        _CACHE["nc"] = _build_nc()
    return _CACHE["nc"]


def make_in_maps(is_electron_logit, true_segmap, binary_mask_logits, portion_logits,
                 incidence_points, positions, chol, occupancy_logits, occupancy_true,
                 matched_q, matched_e):
    f = np.float32
    in_maps = []
    for c in range(8):
        b, h = c // 2, c % 2
        sl = slice(h * HALF, (h + 1) * HALF)
        me = np.asarray(matched_e[b])
        mq = np.asarray(matched_q[b])
        true_sl = np.ascontiguousarray(true_segmap[b, sl], dtype=f).reshape(NPIX, E)
        por_sl = np.ascontiguousarray(portion_logits[b, sl], dtype=f).reshape(NPIX, Q)
        bin_sl = np.ascontiguousarray(binary_mask_logits[b, sl], dtype=f).reshape(NPIX, Q)
        # channel gathers: pure indexing (reference's take_along_axis layout)
        por_pack = np.ascontiguousarray(por_sl[:, mq]).reshape(NCHUNK * P, JC, M)
        tru_pack = np.ascontiguousarray(true_sl[:, me]).reshape(NCHUNK * P, JC, M)
        occ_sl = np.asarray(occupancy_logits[b, sl], dtype=f).reshape(P, J, K)
        occt = np.asarray(occupancy_true[b, sl], dtype=f).reshape(P, J, 1)
        occ_pack = np.concatenate([occ_sl, occt], axis=2)

        sm = np.zeros((P, SC), dtype=f)
        sm[:M, C_RB] = -h * NPIX
        sm[:M, C_INC:C_INC + 2] = np.asarray(incidence_points[b], dtype=f)[me]
        sm[:M, C_DROF:C_DROF + WIN] = np.tile(np.arange(WIN, dtype=f) * W, (M, 1))
        sm[:M, C_POS:C_POS + 2] = np.asarray(positions[b], dtype=f)[mq]
        sm[:M, C_CHOL:C_CHOL + 4] = np.asarray(chol[b], dtype=f).reshape(Q, 4)[mq]
        sm[:M, C_MENP] = me.astype(f) * NPIX
        sm[:M, C_MQNP] = mq.astype(f) * NPIX
        sm[0, C_IEL:C_IEL + Q] = np.asarray(is_electron_logit, dtype=f).reshape(B, Q)[b]
        ind = np.zeros(Q, dtype=f)
        ind[mq] = 1.0
        sm[0, C_IND:C_IND + Q] = ind
        sm[:M, C_I96:C_I96 + M] = np.eye(M, dtype=f)

        def flat_pad(cm):
            out = np.zeros((1, PADF + cm.size + PADB), dtype=f)
            out[0, PADF:PADF + cm.size] = cm.reshape(-1)
            return out

        in_maps.append(dict(
            por_pack=por_pack,
            tru_pack=tru_pack,
            occ_pack=occ_pack,
            true_cm=flat_pad(np.ascontiguousarray(true_sl.T)),
            bin_cm=flat_pad(np.ascontiguousarray(bin_sl.T)),
            smalls=sm,
        ))
    return in_maps


def combine(partials_list):
    s = np.stack([np.asarray(p, dtype=np.float64).reshape(8) for p in partials_list])
    # slots: 0=nll_sum 1=bce_sum 2=num2_sum 3=den_true_sum 4=occ_sum 6=class_sum
    class_loss = s[0::2, 6].sum() / (B * Q)
    nll_loss = s[0::2, 0].sum() / (B * M)
    bce_loss = s[:, 1].sum() / (B * M * WIN * WIN)
    occ_loss = s[:, 4].sum() / (B * H * W)
    dice = 0.0
    for b in range(B):
        num = 2.0 * (s[2 * b, 2] + s[2 * b + 1, 2])
        den = s[2 * b, 3] + s[2 * b + 1, 3] + H * W
        dice += 1.0 - (num + 1.0) / (den + 1.0)
    dice_loss = dice / B
    return np.float32(class_loss + bce_loss + dice_loss + nll_loss + occ_loss)


def kernel(**inputs):
    from concourse.bass_utils import run_bass_kernel_spmd
    nc = _get_nc()
    in_maps = make_in_maps(**{k: np.asarray(v) for k, v in inputs.items()})
    r = run_bass_kernel_spmd(nc, in_maps, list(range(8)))
    return combine([r.results[c]["partials"] for c in range(8)])


# revision 36
# speedup vs baseline: 1.3367x; 1.3367x over previous
"""Trainium2 Bass kernel for nn_Criterion_32830730011569.

8 cores = (image b in 0..3) x (H-half h in 0..1). Host-side prep is pure
indexing/layout (slice, transpose, channel-gather by matched_q/matched_e,
pack) — all arithmetic runs on device.

Each core streams its [96,192] pixel slice once as a packed
[NCHUNK*P, JC, 192] tensor (matched-por || matched-true, chunk-contiguous
1.42MB linear DMAs, staggered prefetch):
  - dice: exp on ACT; softmax denominator is one DVE reduce (channels are
    pre-gathered so no mask is needed); bf16 matmuls accumulate
    C[m1,m2] = (true_r/Z)^T exp_r into PSUM. num = 2*trace(C); den =
    sum of all C entries (each row of exp_r/Z sums to 1).
  - 7x7-window BCE: true/bin shipped channel-major + padded so each m's
    whole window lives in one contiguous 1159-float run; ONE indirect DMA
    per tensor gathers all 96 runs; BCE uses ACT Softplus.
  - occupancy CE: streamed exp + ACT Ln logsumexp + label select.
  - class / NLL: tiny per-query math from host-gathered rows, ACT Ln.
Small setup inputs ride in one packed [128,SC] tensor on the Scalar
engine's HWDGE queue. Each core returns 7 partial sums; the host combines.
"""
import sys

sys.path.insert(0, "/opt/trn_rl_repo")
import numpy as np

B, H, W, Q, E, M, K, WIN = 4, 192, 192, 160, 96, 96, 4, 7
NO_E = 0.1
HALF = H // 2          # rows per core slice
NPIX = HALF * W        # 18432 pixels per slice
P = 128                # partitions
J = NPIX // P          # 144 pixels per partition
NCHUNK = 8
JC = J // NCHUNK       # 18
CH = 2 * M             # 192 packed channels (por_r | true_r)
MAGIC = 8388608.0      # 2^23
MAGIC_I = 0x4B000000
RUNW = 6 * W + 7       # one contiguous span covering a whole 7x7 window
PADF = RUNW            # front pad so straddle-up windows stay row-aligned
PADB = 2400

# smalls column map
C_RB, C_INC, C_DROF, C_POS, C_CHOL, C_MENP, C_MQNP = 0, 1, 3, 10, 12, 16, 17
C_IEL, C_IND, C_I96 = 18, 178, 338
SC = 338 + M

_CACHE = {}


def _build_nc():
    import concourse.bass as bass
    import concourse.bacc as bacc
    import concourse.tile as tile
    from concourse import mybir
    from concourse.tile import add_dep_helper

    f32 = mybir.dt.float32
    i32 = mybir.dt.int32
    bf16 = mybir.dt.bfloat16
    AF = mybir.ActivationFunctionType
    OP = mybir.AluOpType
    AX = mybir.AxisListType

    nc = bacc.Bacc("TRN2", target_bir_lowering=False, debug=False, num_devices=8)

    # ---- external I/O ----
    por_pack = nc.dram_tensor("por_pack", [NCHUNK * P, JC, M], f32, kind="ExternalInput")
    tru_pack = nc.dram_tensor("tru_pack", [NCHUNK * P, JC, M], f32, kind="ExternalInput")
    occ_pack = nc.dram_tensor("occ_pack", [P, J, K + 1], f32, kind="ExternalInput")
    true_cm = nc.dram_tensor("true_cm", [1, PADF + E * NPIX + PADB], f32, kind="ExternalInput")
    bin_cm = nc.dram_tensor("bin_cm", [1, PADF + Q * NPIX + PADB], f32, kind="ExternalInput")
    smalls = nc.dram_tensor("smalls", [P, SC], f32, kind="ExternalInput")
    partials = nc.dram_tensor("partials", [1, 8], f32, kind="ExternalOutput")

    def bc(ap, pos, count):
        """Insert a stride-0 broadcast dim into an AP at free-dim position pos."""
        new = list(ap.ap)
        new.insert(pos, [0, count])
        return bass.AP(tensor=ap.tensor, offset=ap.offset, ap=new)

    from contextlib import ExitStack

    with tile.TileContext(nc) as tc, ExitStack() as ctx:
        sing = ctx.enter_context(tc.tile_pool(name="sing", bufs=1))
        pkp = ctx.enter_context(tc.tile_pool(name="pkp", bufs=3))
        big = ctx.enter_context(tc.tile_pool(name="big", bufs=2))
        ps = ctx.enter_context(tc.tile_pool(name="ps", bufs=1, space="PSUM"))

        # ---------- DMA issues: chunk stream on Sync, smalls/occ on Scalar ----------
        def issue_chunk(c):
            # por on the Sync HWDGE queue set, true on the Scalar set: the two
            # 0.71MB transfers ride different queue sets in parallel
            tp = pkp.tile([P, JC, M], f32, tag="pkp")
            nc.sync.dma_start(out=tp[:], in_=por_pack.ap()[c * P:(c + 1) * P, :, :])
            tt = pkp.tile([P, JC, M], f32, tag="pkt")
            nc.sync.dma_start(out=tt[:], in_=tru_pack.ap()[c * P:(c + 1) * P, :, :])
            return (tp, tt)

        pk_fifo = [issue_chunk(0)]
        sm = sing.tile([P, SC], f32)
        nc.scalar.dma_start(out=sm[:], in_=smalls.ap())
        occ_t = sing.tile([P, J, K + 1], f32)
        nc.scalar.dma_start(out=occ_t[:], in_=occ_pack.ap())
        pk_fifo.append(issue_chunk(1))

        def S(p0, p1, c0, c1):
            return sm[p0:p1, c0:c1]

        ones = sing.tile([P, 1], f32)
        nc.vector.memset(ones[:], 1.0)
        stats = sing.tile([P, 6], f32)
        nc.vector.memset(stats[:], 0.0)
        res = sing.tile([1, 8], f32)
        nc.vector.memset(res[:], 0.0)

        # ---------- window offsets (first DVE work; needs only smalls) ----------
        ptsr = S(0, M, C_INC, C_INC + 2)
        rmag = sing.tile([M, 2], f32)
        nc.vector.tensor_scalar(out=rmag[:], in0=ptsr, scalar1=MAGIC, scalar2=-MAGIC,
                                op0=OP.add, op1=OP.add)
        gtm = sing.tile([M, 2], f32)
        nc.vector.tensor_tensor(out=gtm[:], in0=rmag[:], in1=ptsr, op=OP.is_gt)
        pixf = sing.tile([M, 2], f32)
        nc.vector.tensor_tensor(out=pixf[:], in0=rmag[:], in1=gtm[:], op=OP.subtract)
        base = sing.tile([M, 1], f32)
        nc.vector.tensor_scalar(out=base[:], in0=pixf[:, 0:1], scalar1=float(W),
                                scalar2=float(-3 * W - 3), op0=OP.mult, op1=OP.add)
        nc.vector.tensor_tensor(out=base[:], in0=base[:], in1=pixf[:, 1:2], op=OP.add)
        sofs = sing.tile([M, WIN], f32)
        nc.vector.tensor_scalar(out=sofs[:], in0=S(0, M, C_DROF, C_DROF + WIN),
                                scalar1=base[:], scalar2=S(0, M, C_RB, C_RB + 1),
                                op0=OP.add, op1=OP.add)
        v1 = sing.tile([M, WIN], f32)
        nc.vector.tensor_scalar(out=v1[:], in0=sofs[:], scalar1=0.0, scalar2=None, op0=OP.is_ge)
        v2 = sing.tile([M, WIN], f32)
        nc.vector.tensor_scalar(out=v2[:], in0=sofs[:], scalar1=float(NPIX - 1), scalar2=None, op0=OP.is_le)
        valid = sing.tile([M, WIN], f32)
        nc.vector.tensor_tensor(out=valid[:], in0=v1[:], in1=v2[:], op=OP.mult)
        # one offset per m: start of the contiguous RUNW-float span, clamped so
        # the span stays inside the padded flat tensor. Border margins mean
        # straddling windows are never clamped, so valid rows stay row-aligned.
        clam0 = sing.tile([M, 1], f32)
        nc.vector.tensor_scalar(out=clam0[:], in0=sofs[:, 0:1], scalar1=float(-(6 * W + 4)),
                                scalar2=float(NPIX - WIN), op0=OP.max, op1=OP.min)
        soft = sing.tile([M, 1], f32)
        nc.vector.tensor_scalar(out=soft[:], in0=clam0[:], scalar1=S(0, M, C_MENP, C_MENP + 1),
                                scalar2=MAGIC + PADF, op0=OP.add, op1=OP.add)
        soft_i = sing.tile([M, 1], i32)
        nc.vector.tensor_scalar(out=soft_i[:], in0=soft[:].bitcast(i32), scalar1=0x007FFFFF,
                                scalar2=None, op0=OP.bitwise_and)
        sofb = sing.tile([M, 1], f32)
        nc.vector.tensor_scalar(out=sofb[:], in0=clam0[:], scalar1=S(0, M, C_MQNP, C_MQNP + 1),
                                scalar2=MAGIC + PADF, op0=OP.add, op1=OP.add)
        sofb_i = sing.tile([M, 1], i32)
        nc.vector.tensor_scalar(out=sofb_i[:], in0=sofb[:].bitcast(i32), scalar1=0x007FFFFF,
                                scalar2=None, op0=OP.bitwise_and)

        # ---------- window gathers: one RUNW-float run per matched electron ----------
        tw = sing.tile([M, RUNW], f32)
        bw = sing.tile([M, RUNW], f32)
        true_flat = bass.AP(tensor=true_cm.ap().tensor, offset=0,
                            ap=[[1, PADF + E * NPIX + PADB], [1, 1]])
        bin_flat = bass.AP(tensor=bin_cm.ap().tensor, offset=0,
                           ap=[[1, PADF + Q * NPIX + PADB], [1, 1]])
        nc.gpsimd.indirect_dma_start(
            out=tw[:], out_offset=None, in_=true_flat,
            in_offset=bass.IndirectOffsetOnAxis(ap=soft_i[:], axis=0))
        nc.gpsimd.indirect_dma_start(
            out=bw[:], out_offset=None, in_=bin_flat,
            in_offset=bass.IndirectOffsetOnAxis(ap=sofb_i[:], axis=0))

        # ---------- dice streaming ----------
        C_ps = ps.tile([M, M], f32)
        for c in range(NCHUNK):
            pk_p, pk_tt = pk_fifo.pop(0)
            por_v = pk_p[:]
            tru_v = pk_tt[:]
            exp_t = big.tile([P, JC, M], bf16, tag="exp")
            exp_i = nc.scalar.activation(out=exp_t[:], in_=por_v, func=AF.Exp)
            if c == NCHUNK - 1:
                last_exp = exp_i
            zq_t = big.tile([P, JC], f32, tag="zq")
            nc.vector.reduce_sum(out=zq_t[:], in_=exp_t[:], axis=AX.X)
            rz_t = big.tile([P, JC], bf16, tag="rz")
            with nc.allow_low_precision(reason="rz scales both num and den; error cancels in dice ratio"):
                nc.vector.reciprocal(out=rz_t[:], in_=zq_t[:])
            tc_t = big.tile([P, JC, M], bf16, tag="tc")
            nc.scalar.activation(out=tc_t[:], in_=tru_v, func=AF.Copy)
            a_t = big.tile([P, JC, M], bf16, tag="a")
            nc.vector.tensor_tensor(out=a_t[:], in0=tc_t[:], in1=bc(rz_t[:], 2, M), op=OP.mult)
            for kb in range(JC):
                nc.tensor.matmul(out=C_ps[:], lhsT=a_t[:, kb, :], rhs=exp_t[:, kb, :],
                                 start=(c == 0 and kb == 0),
                                 stop=(c == NCHUNK - 1 and kb == JC - 1))
            if c + 2 < NCHUNK:
                pk_fifo.append(issue_chunk(c + 2))

            # ---- Exp-table / DVE work slotted between chunks (Ln deferred) ----
            if c == 2:
                e4 = sing.tile([P, J, K], f32)
                nc.scalar.activation(out=e4[:], in_=occ_t[:, :, 0:K], func=AF.Exp)
                s4 = sing.tile([P, J], f32)
                nc.vector.reduce_sum(out=s4[:], in_=e4[:], axis=AX.X)
            if c == 3:
                # occupancy CE: label select
                xt = sing.tile([P, J], f32)
                mk = sing.tile([P, J], f32)
                pk2 = sing.tile([P, J], f32)
                for k in range(K):
                    nc.vector.tensor_scalar(out=mk[:], in0=occ_t[:, :, K], scalar1=float(k),
                                            scalar2=None, op0=OP.is_equal)
                    if k == 0:
                        nc.vector.tensor_tensor(out=xt[:], in0=mk[:], in1=occ_t[:, :, 0], op=OP.mult)
                    else:
                        nc.vector.tensor_tensor(out=pk2[:], in0=mk[:], in1=occ_t[:, :, k], op=OP.mult)
                        nc.vector.tensor_tensor(out=xt[:], in0=xt[:], in1=pk2[:], op=OP.add)
            if c == 4:
                # 7x7 window BCE: the 49 window values sit at run[a*W + b]
                def win_ap(t):
                    pdim = t[:].ap[0]
                    return bass.AP(tensor=t[:].tensor, offset=t[:].offset,
                                   ap=[pdim, [W, WIN], [1, WIN]])

                tv = sing.tile([M, WIN * WIN], f32)
                nc.vector.tensor_copy(out=tv[:].rearrange("m (a b) -> m a b", a=WIN),
                                      in_=win_ap(tw))
                lg = sing.tile([M, WIN * WIN], f32)
                nc.vector.tensor_copy(out=lg[:].rearrange("m (a b) -> m a b", a=WIN),
                                      in_=win_ap(bw))
                # softplus = Ln(1 + exp); the Ln half runs in the tail
                exw = sing.tile([M, WIN * WIN], f32)
                nc.scalar.activation(out=exw[:], in_=lg[:], func=AF.Exp)
                nc.vector.tensor_scalar(out=exw[:], in0=exw[:], scalar1=1.0, scalar2=None, op0=OP.add)
                prw = sing.tile([M, WIN * WIN], f32)
                nc.vector.tensor_tensor(out=prw[:], in0=lg[:], in1=tv[:], op=OP.mult)
            if c == 5:
                valid49 = sing.tile([M, WIN * WIN], f32)
                nc.vector.tensor_copy(out=valid49[:].rearrange("m (a b) -> m a b", a=WIN),
                                      in_=bc(valid[:], 2, WIN))
                iel = S(0, 1, C_IEL, C_IEL + Q)
                exc = sing.tile([1, Q], f32)
                nc.scalar.activation(out=exc[:], in_=iel, func=AF.Exp)
                nc.vector.tensor_scalar(out=exc[:], in0=exc[:], scalar1=1.0, scalar2=None, op0=OP.add)
            if c == 6:
                # NLL (96 partitions): everything except the Ln
                cenr = S(0, M, C_POS, C_POS + 2)
                chol0 = S(0, M, C_CHOL, C_CHOL + 1)
                chol1 = S(0, M, C_CHOL + 2, C_CHOL + 3)
                chol3 = S(0, M, C_CHOL + 3, C_CHOL + 4)
                d_ = sing.tile([M, 2], f32)
                nc.vector.tensor_tensor(out=d_[:], in0=ptsr, in1=cenr, op=OP.subtract)
                r00 = sing.tile([M, 1], f32)
                nc.vector.reciprocal(out=r00[:], in_=chol0)
                r11 = sing.tile([M, 1], f32)
                nc.vector.reciprocal(out=r11[:], in_=chol3)
                z0 = sing.tile([M, 1], f32)
                nc.vector.tensor_tensor(out=z0[:], in0=d_[:, 0:1], in1=r00[:], op=OP.mult)
                t1 = sing.tile([M, 1], f32)
                nc.vector.tensor_tensor(out=t1[:], in0=chol1, in1=z0[:], op=OP.mult)
                nc.vector.tensor_tensor(out=t1[:], in0=d_[:, 1:2], in1=t1[:], op=OP.subtract)
                z1 = sing.tile([M, 1], f32)
                nc.vector.tensor_tensor(out=z1[:], in0=t1[:], in1=r11[:], op=OP.mult)
                sq = sing.tile([M, 1], f32)
                nc.vector.tensor_tensor(out=sq[:], in0=z0[:], in1=z0[:], op=OP.mult)
                sq1 = sing.tile([M, 1], f32)
                nc.vector.tensor_tensor(out=sq1[:], in0=z1[:], in1=z1[:], op=OP.mult)
                nc.vector.tensor_tensor(out=sq[:], in0=sq[:], in1=sq1[:], op=OP.add)
                ldet = sing.tile([M, 1], f32)
                nc.vector.tensor_tensor(out=ldet[:], in0=chol0, in1=chol3, op=OP.mult)
                nc.vector.tensor_scalar(out=sq[:], in0=sq[:], scalar1=0.5,
                                        scalar2=float(np.log(2.0 * np.pi)), op0=OP.mult, op1=OP.add)

        # ---------- tail: all Ln work (one ACT table switch), pinned last ----------
        lse = sing.tile([P, J], f32)
        ln_i = nc.scalar.activation(out=lse[:], in_=s4[:], func=AF.Ln)
        add_dep_helper(ln_i.ins, last_exp.ins, reason="one table switch at tail")
        spw = sing.tile([M, WIN * WIN], f32)
        ln_w = nc.scalar.activation(out=spw[:], in_=exw[:], func=AF.Ln)
        add_dep_helper(ln_w.ins, last_exp.ins, reason="one table switch at tail")
        sp = sing.tile([1, Q], f32)
        ln_c = nc.scalar.activation(out=sp[:], in_=exc[:], func=AF.Ln)
        add_dep_helper(ln_c.ins, last_exp.ins, reason="one table switch at tail")
        lnd = sing.tile([M, 1], f32)
        ln_n = nc.scalar.activation(out=lnd[:], in_=ldet[:], func=AF.Ln)
        add_dep_helper(ln_n.ins, last_exp.ins, reason="one table switch at tail")

        # occupancy CE finish
        nc.vector.tensor_tensor(out=lse[:], in0=lse[:], in1=xt[:], op=OP.subtract)
        nc.vector.reduce_sum(out=stats[:, 4:5], in_=lse[:], axis=AX.X)
        # window BCE finish
        nc.vector.tensor_tensor(out=spw[:], in0=spw[:], in1=prw[:], op=OP.subtract)
        scr_w = sing.tile([M, WIN * WIN], f32)
        nc.vector.tensor_tensor(out=scr_w[:], in0=spw[:], in1=valid49[:], op=OP.mult)
        nc.vector.reduce_sum(out=stats[0:M, 1:2], in_=scr_w[:], axis=AX.X)
        # class finish
        iel = S(0, 1, C_IEL, C_IEL + Q)
        ind1 = S(0, 1, C_IND, C_IND + Q)
        t9 = sing.tile([1, Q], f32)
        nc.vector.tensor_scalar(out=t9[:], in0=sp[:], scalar1=0.9, scalar2=None, op0=OP.mult)
        nc.vector.tensor_tensor(out=t9[:], in0=t9[:], in1=iel, op=OP.subtract)
        scr_q = sing.tile([1, Q], f32)
        clsm = sing.tile([1, 1], f32)
        nc.vector.tensor_tensor(out=scr_q[:], in0=t9[:], in1=ind1, op=OP.mult)
        nc.vector.reduce_sum(out=clsm[:], in_=scr_q[:], axis=AX.X)
        spsum = sing.tile([1, 1], f32)
        nc.vector.reduce_sum(out=spsum[:], in_=sp[:], axis=AX.X)
        nc.vector.tensor_scalar(out=spsum[:], in0=spsum[:], scalar1=NO_E, scalar2=None, op0=OP.mult)
        nc.vector.tensor_tensor(out=res[:, 6:7], in0=spsum[:], in1=clsm[:], op=OP.add)
        # NLL finish
        nc.vector.tensor_tensor(out=stats[0:M, 0:1], in0=sq[:], in1=lnd[:], op=OP.add)

        # ---------- dice epilogue ----------
        Cs = sing.tile([M, M], f32)
        nc.vector.tensor_copy(out=Cs[:], in_=C_ps[:])
        # rhs rows (exp_r/Z) sum to 1, so summing all of C gives sum(true): den.
        nc.vector.reduce_sum(out=stats[0:M, 3:4], in_=Cs[:], axis=AX.X)
        scr_c = sing.tile([M, M], f32)
        nc.vector.tensor_tensor(out=scr_c[:], in0=Cs[:], in1=S(0, M, C_I96, C_I96 + M), op=OP.mult)
        nc.vector.reduce_sum(out=stats[0:M, 2:3], in_=scr_c[:], axis=AX.X)

        # ---------- final cross-partition reduction ----------
        fin_ps = ps.tile([1, 6], f32)
        nc.tensor.matmul(out=fin_ps[:], lhsT=ones[:], rhs=stats[:], start=True, stop=True)
        nc.vector.tensor_copy(out=res[:, 0:6], in_=fin_ps[:])
        nc.sync.dma_start(out=partials.ap(), in_=res[:])

    nc.compile()
    return nc


def _get_nc():
    if "nc" not in _CACHE:
        _CACHE["nc"] = _build_nc()
    return _CACHE["nc"]


def make_in_maps(is_electron_logit, true_segmap, binary_mask_logits, portion_logits,
                 incidence_points, positions, chol, occupancy_logits, occupancy_true,
                 matched_q, matched_e):
    f = np.float32
    in_maps = []
    for c in range(8):
        b, h = c // 2, c % 2
        sl = slice(h * HALF, (h + 1) * HALF)
        me = np.asarray(matched_e[b])
        mq = np.asarray(matched_q[b])
        true_sl = np.ascontiguousarray(true_segmap[b, sl], dtype=f).reshape(NPIX, E)
        por_sl = np.ascontiguousarray(portion_logits[b, sl], dtype=f).reshape(NPIX, Q)
        bin_sl = np.ascontiguousarray(binary_mask_logits[b, sl], dtype=f).reshape(NPIX, Q)
        # channel gathers: pure indexing (reference's take_along_axis layout)
        por_pack = np.ascontiguousarray(por_sl[:, mq]).reshape(NCHUNK * P, JC, M)
        tru_pack = np.ascontiguousarray(true_sl[:, me]).reshape(NCHUNK * P, JC, M)
        occ_sl = np.asarray(occupancy_logits[b, sl], dtype=f).reshape(P, J, K)
        occt = np.asarray(occupancy_true[b, sl], dtype=f).reshape(P, J, 1)
        occ_pack = np.concatenate([occ_sl, occt], axis=2)

        sm = np.zeros((P, SC), dtype=f)
        sm[:M, C_RB] = -h * NPIX
        sm[:M, C_INC:C_INC + 2] = np.asarray(incidence_points[b], dtype=f)[me]
        sm[:M, C_DROF:C_DROF + WIN] = np.tile(np.arange(WIN, dtype=f) * W, (M, 1))
        sm[:M, C_POS:C_POS + 2] = np.asarray(positions[b], dtype=f)[mq]
        sm[:M, C_CHOL:C_CHOL + 4] = np.asarray(chol[b], dtype=f).reshape(Q, 4)[mq]
        sm[:M, C_MENP] = me.astype(f) * NPIX
        sm[:M, C_MQNP] = mq.astype(f) * NPIX
        sm[0, C_IEL:C_IEL + Q] = np.asarray(is_electron_logit, dtype=f).reshape(B, Q)[b]
        ind = np.zeros(Q, dtype=f)
        ind[mq] = 1.0
        sm[0, C_IND:C_IND + Q] = ind
        sm[:M, C_I96:C_I96 + M] = np.eye(M, dtype=f)

        def flat_pad(cm):
            out = np.zeros((1, PADF + cm.size + PADB), dtype=f)
            out[0, PADF:PADF + cm.size] = cm.reshape(-1)
            return out

        in_maps.append(dict(
            por_pack=por_pack,
            tru_pack=tru_pack,
            occ_pack=occ_pack,
            true_cm=flat_pad(np.ascontiguousarray(true_sl.T)),
            bin_cm=flat_pad(np.ascontiguousarray(bin_sl.T)),
            smalls=sm,
        ))
    return in_maps


def combine(partials_list):
    s = np.stack([np.asarray(p, dtype=np.float64).reshape(8) for p in partials_list])
    # slots: 0=nll_sum 1=bce_sum 2=num2_sum 3=den_true_sum 4=occ_sum 6=class_sum
    class_loss = s[0::2, 6].sum() / (B * Q)
    nll_loss = s[0::2, 0].sum() / (B * M)
    bce_loss = s[:, 1].sum() / (B * M * WIN * WIN)
    occ_loss = s[:, 4].sum() / (B * H * W)
    dice = 0.0
    for b in range(B):
        num = 2.0 * (s[2 * b, 2] + s[2 * b + 1, 2])
        den = s[2 * b, 3] + s[2 * b + 1, 3] + H * W
        dice += 1.0 - (num + 1.0) / (den + 1.0)
    dice_loss = dice / B
    return np.float32(class_loss + bce_loss + dice_loss + nll_loss + occ_loss)


def kernel(**inputs):
    from concourse.bass_utils import run_bass_kernel_spmd
    nc = _get_nc()
    in_maps = make_in_maps(**{k: np.asarray(v) for k, v in inputs.items()})
    r = run_bass_kernel_spmd(nc, in_maps, list(range(8)))
    return combine([r.results[c]["partials"] for c in range(8)])
